# revision 1
# baseline (speedup 1.0000x reference)
"""CategoricalLstmDecoder Trainium kernel (self-contained).

8-way gate-split tensor parallel across 8 NeuronCores; f32r matmuls;
h exchanged via AllGather; sigmoid via tanh identity.

Layout notes (per core c of 8):
- Gate chunk order per core: [i(128), f(128), o(128), g(128)] rows, where
  i/f/o rows are NEGATED host-side (sigmoid(x) = 1/(1+exp(-x)) via exp).
- All matmuls f32r (fp32 storage, bf16-class speed).
- h exchanged as h.T chunks [128 hdim, 64 batch] via ncfw AllGather.
"""
import sys
sys.path.insert(0, "/opt/trn_rl_repo")
import numpy as np
import concourse.bass as bass
import concourse.tile as tile
from concourse import bacc, mybir

N = 8
B = 64
HID = 1024
VOCAB = 512
CH = HID // N  # 128 h-dims per core
GW = 4 * CH    # 512 gate rows per core
f32 = mybir.dt.float32
f32r = mybir.dt.float32r
AFT = mybir.ActivationFunctionType


def host_prep(z, fc_w, fc_b, w_ih0, w_hh0, b_ih0, b_hh0,
              w_ih1, w_hh1, b_ih1, b_hh1, out_w, out_b):
    """Build per-core input dicts. All fp32 numpy."""
    z = np.asarray(z, np.float32)
    h_init = np.tanh(z @ np.asarray(fc_w, np.float32).T + np.asarray(fc_b, np.float32))  # [B, HID]
    # hT layout [128, 8, 64]: [p, j, b] = h_init[b, 128*j + p]
    hT_init = np.transpose(h_init.reshape(B, N, CH), (2, 1, 0)).copy()  # [CH, N, B]

    b0 = np.asarray(b_ih0, np.float32) + np.asarray(b_hh0, np.float32)
    b1 = np.asarray(b_ih1, np.float32) + np.asarray(b_hh1, np.float32)

    def chunk_rows(c):
        # rows of the 4H gate matrix owned by core c, in [i, f, o, g] order
        i0 = np.arange(c * CH, (c + 1) * CH)
        return np.concatenate([i0, HID + i0, 3 * HID + i0, 2 * HID + i0])

    def prep_w(w, c):
        # returns W.T chunk [K, GW] with i/f/o negated, as [128, K//128, GW]
        w = np.asarray(w, np.float32)
        rows = w[chunk_rows(c)]  # [GW, K]
        sgn = np.ones((GW, 1), np.float32)
        sgn[: 3 * CH] = 0.5
        rows = rows * sgn
        K = rows.shape[1]
        return np.ascontiguousarray(
            rows.T.reshape(K // 128, 128, GW).transpose(1, 0, 2))  # [128, K/128, GW]

    def prep_b(b, c):
        bb = b[chunk_rows(c)].astype(np.float32).copy()
        bb[: 3 * CH] *= 0.5
        return bb.reshape(1, GW)

    out_wT = np.ascontiguousarray(
        np.asarray(out_w, np.float32).T.reshape(N, CH, VOCAB).transpose(1, 0, 2))  # [128, 8, 512]

    in_maps = []
    for c in range(N):
        in_maps.append({
            "hT_init": hT_init,
            "wih0": prep_w(w_ih0, c),
            "whh0": prep_w(w_hh0, c),
            "wih1": prep_w(w_ih1, c),
            "whh1": prep_w(w_hh1, c),
            "outw": out_wT,
            "b0": prep_b(b0, c),
            "b1": prep_b(b1, c),
            "outb": np.asarray(out_b, np.float32).reshape(1, VOCAB).copy(),
            "ones": np.ones((1, B), np.float32),
            "eye": np.eye(B, dtype=np.float32),
        })
    return in_maps


def build_kernel(T):
    nc = bacc.Bacc("TRN2", target_bir_lowering=False, debug=False, num_devices=N)
    dp = nc.declare_dram_parameter
    hT_init_d = dp("hT_init", [CH, N, B], f32, isOutput=False)
    wih0_d = dp("wih0", [128, 4, GW], f32, isOutput=False)
    whh0_d = dp("whh0", [128, 8, GW], f32, isOutput=False)
    wih1_d = dp("wih1", [128, 8, GW], f32, isOutput=False)
    whh1_d = dp("whh1", [128, 8, GW], f32, isOutput=False)
    outw_d = dp("outw", [128, 8, VOCAB], f32, isOutput=False)
    b0_d = dp("b0", [1, GW], f32, isOutput=False)
    b1_d = dp("b1", [1, GW], f32, isOutput=False)
    outb_d = dp("outb", [1, VOCAB], f32, isOutput=False)
    ones_d = dp("ones", [1, B], f32, isOutput=False)
    eye_d = dp("eye", [B, B], f32, isOutput=False)
    out_d = dp("out", [T, B, VOCAB], f32, isOutput=True)

    with tile.TileContext(nc) as tc:
        with (
            tc.tile_pool(name="wpool", bufs=1) as wpool,
            tc.tile_pool(name="state", bufs=1) as state,
            tc.tile_pool(name="sp", bufs=2) as sp,
            tc.tile_pool(name="ps", bufs=1, space=bass.MemorySpace.PSUM) as ps,
            tc.tile_pool(name="dram", bufs=2, space="DRAM") as dram,
        ):
            # ---- load + round weights (one-time) ----
            def load_round(dram_t, shape, name):
                stage = sp.tile(shape, f32, tag="wstage")
                nc.sync.dma_start(stage[:], dram_t[:])
                wr = wpool.tile(shape, f32r, tag=f"w_{name}")
                nc.vector.tensor_copy(wr[:], stage[:])
                return wr

            wih0 = load_round(wih0_d, [128, 4, GW], "wih0")
            whh0 = load_round(whh0_d, [128, 8, GW], "whh0")
            wih1 = load_round(wih1_d, [128, 8, GW], "wih1")
            whh1 = load_round(whh1_d, [128, 8, GW], "whh1")
            outw = load_round(outw_d, [128, 8, VOCAB], "outw")
            b0 = load_round(b0_d, [1, GW], "b0")
            b1 = load_round(b1_d, [1, GW], "b1")
            outb = load_round(outb_d, [1, VOCAB], "outb")
            ones = load_round(ones_d, [1, B], "ones")
            eye = wpool.tile([B, B], f32, tag="eye")
            nc.sync.dma_start(eye[:], eye_d[:])
            hT0 = load_round(hT_init_d, [CH, N, B], "hTi")  # gathered h0.T (f32r)
            hT1 = hT0  # same initial data for both layers

            c0 = state.tile([B, CH], f32)
            c1 = state.tile([B, CH], f32)
            nc.vector.memset(c0[:], 0.0)
            nc.vector.memset(c1[:], 0.0)

            xT = None  # [128, 4, B] f32r; None at t=0 (x0 = 0)

            def lstm_layer(t, li, g, cstate):
                """gates psum g [B, GW] -> returns h.T sbuf tile [128, B] (f32)."""
                th = sp.tile([B, GW], f32, tag=f"th{li}")
                nc.scalar.activation(th[:], g[:], AFT.Tanh)
                sg = sp.tile([B, 3 * CH], f32, tag=f"sg{li}")
                # sigmoid(x) = 0.5*tanh(x/2)+0.5 (x/2 pre-folded into weights)
                nc.vector.tensor_scalar(sg[:], th[:, 0:3 * CH], 0.5, 0.5,
                                        mybir.AluOpType.mult, mybir.AluOpType.add)
                tg = th[:, 3 * CH:GW]
                t1 = sp.tile([B, CH], f32, tag=f"t1{li}")
                nc.vector.tensor_mul(t1[:], sg[:, 0:CH], tg)          # sig(i)*tanh(g)
                nc.vector.tensor_mul(cstate[:], cstate[:], sg[:, CH:2 * CH])  # c *= sig(f)
                nc.vector.tensor_add(cstate[:], cstate[:], t1[:])
                tc_ = sp.tile([B, CH], f32, tag=f"tc{li}")
                nc.scalar.activation(tc_[:], cstate[:], AFT.Tanh)
                h = sp.tile([B, CH], f32, tag=f"h{li}")
                nc.vector.tensor_mul(h[:], sg[:, 2 * CH:3 * CH], tc_[:])  # sig(o)*tanh(c)
                zhT = ps.tile([CH, B], f32, tag=f"zhT{li}")
                nc.tensor.transpose(zhT[:], h[:], eye[:])
                hTs = sp.tile([CH, B], f32, tag=f"hTs{li}")
                nc.vector.tensor_copy(hTs[:], zhT[:])
                return hTs

            def exchange(t, li, hTs):
                """AllGather h.T chunk -> [128, 8, B] f32r tile."""
                cin = dram.tile([CH, B], f32, tag=f"cin{li}")
                gout = dram.tile([N * CH, B], f32, tag=f"gout{li}")
                nc.sync.dma_start(cin[:], hTs[:])
                nc.gpsimd.collective_compute(
                    "AllGather", mybir.AluOpType.bypass,
                    replica_groups=[list(range(N))],
                    ins=[cin.opt()], outs=[gout.opt()],
                )
                hg = sp.tile([CH, N, B], f32, tag=f"hg{li}")
                src = gout.opt().rearrange("(c p) b -> p c b", p=CH)
                nc.sync.dma_start(hg[:, 0:4, :], src[:, 0:4, :])
                nc.sync.dma_start(hg[:, 4:8, :], src[:, 4:8, :])
                hgr = sp.tile([CH, N, B], f32r, tag=f"hgr{li}")
                nc.vector.tensor_copy(hgr[:, 0:4, :], hg[:, 0:4, :])
                nc.vector.tensor_copy(hgr[:, 4:8, :], hg[:, 4:8, :])
                return hgr

            for t in range(T):
                sc = nc.named_scope if t == T // 2 else None
                # ---- layer 0 gates ----
                g0 = ps.tile([B, GW], f32, tag="g0")
                nc.tensor.matmul(g0[:], ones[:], b0[:], start=True, stop=False)
                for j in range(8):
                    nc.tensor.matmul(g0[:], hT0[:, j, :], whh0[:, j, :],
                                     start=False, stop=(xT is None and j == 7))
                if xT is not None:
                    for j in range(4):
                        nc.tensor.matmul(g0[:], xT[:, j, :], wih0[:, j, :],
                                         start=False, stop=(j == 3))
                if sc:
                    with sc("ew0"):
                        h0Ts = lstm_layer(t, 0, g0, c0)
                    with sc("ex0"):
                        hT0_new = exchange(t, 0, h0Ts)
                else:
                    h0Ts = lstm_layer(t, 0, g0, c0)
                    hT0_new = exchange(t, 0, h0Ts)

                # ---- layer 1 gates ----
                g1 = ps.tile([B, GW], f32, tag="g1")
                nc.tensor.matmul(g1[:], ones[:], b1[:], start=True, stop=False)
                for j in range(8):
                    nc.tensor.matmul(g1[:], hT1[:, j, :], whh1[:, j, :],
                                     start=False, stop=False)
                for j in range(8):
                    nc.tensor.matmul(g1[:], hT0_new[:, j, :], wih1[:, j, :],
                                     start=False, stop=(j == 7))
                if sc:
                    with sc("ew1"):
                        h1Ts = lstm_layer(t, 1, g1, c1)
                    with sc("ex1"):
                        hT1_new = exchange(t, 1, h1Ts)
                else:
                    h1Ts = lstm_layer(t, 1, g1, c1)
                    hT1_new = exchange(t, 1, h1Ts)

                # ---- logits ----
                lg = ps.tile([B, VOCAB], f32, tag="lg")
                nc.tensor.matmul(lg[:], ones[:], outb[:], start=True, stop=False)
                for j in range(8):
                    nc.tensor.matmul(lg[:], hT1_new[:, j, :], outw[:, j, :],
                                     start=False, stop=(j == 7))
                lgs = sp.tile([B, VOCAB], f32, tag="lgs")
                nc.scalar.copy(lgs[:], lg[:])
                nc.gpsimd.dma_start(out_d[t], lgs[:])

                # ---- softmax -> xT (skip at last step) ----
                if t + 1 < T:
                    ex = sp.tile([B, VOCAB], f32r, tag="ex")
                    sums = sp.tile([B, 1], f32, tag="sums")
                    nc.scalar.activation(ex[:], lg[:], AFT.Exp, accum_out=sums[:])
                    rr = sp.tile([B, 1], f32, tag="rr")
                    nc.vector.reciprocal(rr[:], sums[:])
                    dg = sp.tile([B, B], f32r, tag="dg")
                    nc.vector.tensor_scalar_mul(dg[:], eye[:], rr[:])
                    zxT = ps.tile([128, 4, B], f32, tag="zxT")
                    for j in range(4):
                        nc.tensor.matmul(zxT[:, j, :], ex[:, 128 * j:128 * (j + 1)],
                                         dg[:], start=True, stop=True)
                    xT_new = sp.tile([128, 4, B], f32r, tag="xT")
                    nc.vector.tensor_copy(xT_new[:], zxT[:])
                    xT = xT_new

                hT0 = hT0_new
                hT1 = hT1_new


    nc.compile()
    return nc




T_STEPS = 512


def kernel(**inputs):
    import numpy as np
    from concourse import bass_utils
    in_maps = host_prep(**{k: np.asarray(v) for k, v in inputs.items()})
    nc = build_kernel(T_STEPS)
    res = bass_utils.run_bass_kernel_spmd(
        nc, in_maps, core_ids=list(range(N)), trace=False)
    logits = res.results[0]["out"]  # [T, B, V] fp32
    return np.ascontiguousarray(
        np.transpose(logits, (1, 0, 2))[:, :, None, :].astype(np.float32))



# revision 5
# speedup vs baseline: 1.0554x; 1.0554x over previous
"""CategoricalLstmDecoder Trainium kernel (self-contained).

8-way gate-split tensor parallel across 8 NeuronCores; f32r matmuls;
h exchanged via AllGather; sigmoid via tanh identity.

Software-pipelined step: the whh matmuls of the next gate group are
issued while the AllGather of the other layer's h is in flight, so the
PE never sits idle across a collective. Exchange payload travels as
f32r end-to-end (no post-gather casts); AllGather output lives in the
Shared scratchpad (fast HBM-HBM path).

Layout notes (per core c of 8):
- Gate chunk order per core: [i(128), f(128), o(128), g(128)] rows, where
  i/f/o rows are pre-scaled by 0.5 host-side (sigmoid(x)=0.5*tanh(x/2)+0.5).
- h exchanged as h.T chunks [128 hdim, 64 batch] via ncfw AllGather.
"""
import sys
sys.path.insert(0, "/opt/trn_rl_repo")
import numpy as np
import concourse.bass as bass
import concourse.tile as tile
from concourse import bacc, mybir

N = 8
B = 64
HID = 1024
VOCAB = 512
CH = HID // N  # 128 h-dims per core
GW = 4 * CH    # 512 gate rows per core
f32 = mybir.dt.float32
f32r = mybir.dt.float32r
AFT = mybir.ActivationFunctionType


def host_prep(z, fc_w, fc_b, w_ih0, w_hh0, b_ih0, b_hh0,
              w_ih1, w_hh1, b_ih1, b_hh1, out_w, out_b):
    """Build per-core input dicts. All fp32 numpy."""
    z = np.asarray(z, np.float32)
    h_init = np.tanh(z @ np.asarray(fc_w, np.float32).T + np.asarray(fc_b, np.float32))  # [B, HID]
    # hT layout [128, 8, 64]: [p, j, b] = h_init[b, 128*j + p]
    hT_init = np.transpose(h_init.reshape(B, N, CH), (2, 1, 0)).copy()  # [CH, N, B]

    b0 = np.asarray(b_ih0, np.float32) + np.asarray(b_hh0, np.float32)
    b1 = np.asarray(b_ih1, np.float32) + np.asarray(b_hh1, np.float32)

    def chunk_rows(c):
        # rows of the 4H gate matrix owned by core c, in [i, f, o, g] order
        i0 = np.arange(c * CH, (c + 1) * CH)
        return np.concatenate([i0, HID + i0, 3 * HID + i0, 2 * HID + i0])

    def prep_w(w, c):
        # returns W.T chunk [K, GW] with i/f/o scaled 0.5, as [128, K//128, GW]
        w = np.asarray(w, np.float32)
        rows = w[chunk_rows(c)]  # [GW, K]
        sgn = np.ones((GW, 1), np.float32)
        sgn[: 3 * CH] = 0.5
        rows = rows * sgn
        K = rows.shape[1]
        return np.ascontiguousarray(
            rows.T.reshape(K // 128, 128, GW).transpose(1, 0, 2))  # [128, K/128, GW]

    def prep_b(b, c):
        bb = b[chunk_rows(c)].astype(np.float32).copy()
        bb[: 3 * CH] *= 0.5
        return bb.reshape(1, GW)

    out_wT = np.ascontiguousarray(
        np.asarray(out_w, np.float32).T.reshape(N, CH, VOCAB).transpose(1, 0, 2))  # [128, 8, 512]

    in_maps = []
    for c in range(N):
        in_maps.append({
            "hT_init": hT_init,
            "wih0": prep_w(w_ih0, c),
            "whh0": prep_w(w_hh0, c),
            "wih1": prep_w(w_ih1, c),
            "whh1": prep_w(w_hh1, c),
            "outw": out_wT,
            "b0": prep_b(b0, c),
            "b1": prep_b(b1, c),
            "outb": np.asarray(out_b, np.float32).reshape(1, VOCAB).copy(),
            "ones": np.ones((1, B), np.float32),
            "eye": np.eye(B, dtype=np.float32),
        })
    return in_maps


def build_kernel(T):
    nc = bacc.Bacc("TRN2", target_bir_lowering=False, debug=False, num_devices=N)
    dp = nc.declare_dram_parameter
    hT_init_d = dp("hT_init", [CH, N, B], f32, isOutput=False)
    wih0_d = dp("wih0", [128, 4, GW], f32, isOutput=False)
    whh0_d = dp("whh0", [128, 8, GW], f32, isOutput=False)
    wih1_d = dp("wih1", [128, 8, GW], f32, isOutput=False)
    whh1_d = dp("whh1", [128, 8, GW], f32, isOutput=False)
    outw_d = dp("outw", [128, 8, VOCAB], f32, isOutput=False)
    b0_d = dp("b0", [1, GW], f32, isOutput=False)
    b1_d = dp("b1", [1, GW], f32, isOutput=False)
    outb_d = dp("outb", [1, VOCAB], f32, isOutput=False)
    ones_d = dp("ones", [1, B], f32, isOutput=False)
    eye_d = dp("eye", [B, B], f32, isOutput=False)
    out_d = dp("out", [T, B, VOCAB], f32, isOutput=True)

    with tile.TileContext(nc) as tc:
        with (
            tc.tile_pool(name="wpool", bufs=1) as wpool,
            tc.tile_pool(name="state", bufs=1) as state,
            tc.tile_pool(name="sp", bufs=2) as sp,
            tc.tile_pool(name="ps", bufs=1, space=bass.MemorySpace.PSUM) as ps,
            tc.tile_pool(name="dram", bufs=2, space="DRAM") as dram,
        ):
            # ---- load + round weights (one-time) ----
            def load_round(dram_t, shape, name):
                stage = sp.tile(shape, f32, tag="wstage")
                nc.sync.dma_start(stage[:], dram_t[:])
                wr = wpool.tile(shape, f32r, tag=f"w_{name}")
                nc.vector.tensor_copy(wr[:], stage[:])
                return wr

            wih0 = load_round(wih0_d, [128, 4, GW], "wih0")
            whh0 = load_round(whh0_d, [128, 8, GW], "whh0")
            wih1 = load_round(wih1_d, [128, 8, GW], "wih1")
            whh1 = load_round(whh1_d, [128, 8, GW], "whh1")
            outw = load_round(outw_d, [128, 8, VOCAB], "outw")
            b0 = load_round(b0_d, [1, GW], "b0")
            b1 = load_round(b1_d, [1, GW], "b1")
            outb = load_round(outb_d, [1, VOCAB], "outb")
            ones = load_round(ones_d, [1, B], "ones")
            eye = wpool.tile([B, B], f32, tag="eye")
            nc.sync.dma_start(eye[:], eye_d[:])
            hTi = load_round(hT_init_d, [CH, N, B], "hTi")  # full h_init.T (f32r)

            c0 = state.tile([B, CH], f32)
            c1 = state.tile([B, CH], f32)
            nc.vector.memset(c0[:], 0.0)
            nc.vector.memset(c1[:], 0.0)

            # ---- helpers -------------------------------------------------
            def ew(li, g, cstate):
                """gates psum g [B, GW] + cell state -> h sbuf tile [B, CH]."""
                th = sp.tile([B, GW], f32, tag=f"th{li}")
                nc.scalar.activation(th[:], g[:], AFT.Tanh)
                sg = sp.tile([B, 3 * CH], f32, tag=f"sg{li}")
                # sigmoid(x) = 0.5*tanh(x/2)+0.5 (x/2 pre-folded into weights)
                nc.vector.tensor_scalar(sg[:], th[:, 0:3 * CH], 0.5, 0.5,
                                        mybir.AluOpType.mult, mybir.AluOpType.add)
                tg = th[:, 3 * CH:GW]
                t1 = sp.tile([B, CH], f32, tag=f"t1{li}")
                nc.vector.tensor_mul(t1[:], sg[:, 0:CH], tg)          # sig(i)*tanh(g)
                nc.vector.tensor_mul(cstate[:], cstate[:], sg[:, CH:2 * CH])  # c *= sig(f)
                nc.vector.tensor_add(cstate[:], cstate[:], t1[:])
                tc_ = sp.tile([B, CH], f32, tag=f"tc{li}")
                nc.scalar.activation(tc_[:], cstate[:], AFT.Tanh)
                h = sp.tile([B, CH], f32, tag=f"h{li}")
                nc.vector.tensor_mul(h[:], sg[:, 2 * CH:3 * CH], tc_[:])  # sig(o)*tanh(c)
                return h

            def send(li, h):
                """transpose h -> f32r chunk, DMA to DRAM, kick AllGather."""
                zhT = ps.tile([CH, B], f32, tag=f"zhT{li}")
                nc.tensor.transpose(zhT[:], h[:], eye[:])
                hTs = sp.tile([CH, B], f32r, tag=f"hTs{li}")
                nc.vector.tensor_copy(hTs[:], zhT[:])
                cin = dram.tile([CH, B], f32r, tag=f"cin{li}")
                gout = dram.tile([N * CH, B], f32r, tag=f"gout{li}",
                                 addr_space="Shared")
                nc.sync.dma_start(cin[:], hTs[:])
                nc.gpsimd.collective_compute(
                    "AllGather", mybir.AluOpType.bypass,
                    replica_groups=[list(range(N))],
                    ins=[cin.opt()], outs=[gout.opt()],
                )
                return gout

            def recv(li, gout):
                """land the gathered h.T as an f32r [CH, N, B] sbuf tile."""
                hg = sp.tile([CH, N, B], f32r, tag=f"hg{li}")
                src = gout.opt().rearrange("(c p) b -> p c b", p=CH)
                nc.sync.dma_start(hg[:, 0:4, :], src[:, 0:4, :])
                nc.sync.dma_start(hg[:, 4:8, :], src[:, 4:8, :])
                return hg

            # ---- prologue: step 0 layer 0 (x(0)=0, h0(-1)=h_init) -------
            g0 = ps.tile([B, GW], f32, tag="g0")
            nc.tensor.matmul(g0[:], ones[:], b0[:], start=True, stop=False)
            for j in range(N):
                nc.tensor.matmul(g0[:], hTi[:, j, :], whh0[:, j, :],
                                 start=False, stop=(j == N - 1))
            h0 = ew(0, g0, c0)
            ag0 = send(0, h0)
            # A(0): layer-1 recurrent part vs h1(-1)=h_init, overlaps AG0
            g1 = ps.tile([B, GW], f32, tag="g1")
            nc.tensor.matmul(g1[:], ones[:], b1[:], start=True, stop=False)
            for j in range(N):
                nc.tensor.matmul(g1[:], hTi[:, j, :], whh1[:, j, :],
                                 start=False, stop=False)

            for k in range(T):
                last = (k + 1 == T)
                # ---- B: land AG0(k); finish g1 with wih1 @ h0T(k) -------
                hg0 = recv(0, ag0)
                for j in range(N):
                    nc.tensor.matmul(g1[:], hg0[:, j, :], wih1[:, j, :],
                                     start=False, stop=(j == N - 1))
                # ---- C1: ew for layer 1 ---------------------------------
                h1 = ew(1, g1, c1)
                # ---- D1: start g0(k+1) partial (overlaps ew1) -----------
                if not last:
                    g0 = ps.tile([B, GW], f32, tag="g0")
                    nc.tensor.matmul(g0[:], ones[:], b0[:], start=True, stop=False)
                    for j in range(4):
                        nc.tensor.matmul(g0[:], hg0[:, j, :], whh0[:, j, :],
                                         start=False, stop=False)
                # ---- C2: send h1, kick AG1(k) ---------------------------
                ag1 = send(1, h1)
                # ---- D2: rest of whh0 (overlaps AG1) --------------------
                if not last:
                    for j in range(4, N):
                        nc.tensor.matmul(g0[:], hg0[:, j, :], whh0[:, j, :],
                                         start=False, stop=False)
                # ---- E: land AG1(k); logits -----------------------------
                hg1 = recv(1, ag1)
                lg = ps.tile([B, VOCAB], f32, tag="lg")
                nc.tensor.matmul(lg[:], ones[:], outb[:], start=True, stop=False)
                for j in range(N):
                    nc.tensor.matmul(lg[:], hg1[:, j, :], outw[:, j, :],
                                     start=False, stop=(j == N - 1))
                lgs = sp.tile([B, VOCAB], f32, tag="lgs")
                nc.scalar.copy(lgs[:], lg[:])
                nc.gpsimd.dma_start(out_d[k], lgs[:])

                if last:
                    break

                # ---- F: softmax -> xT (f32r [128, 4, B]) ----------------
                ex = sp.tile([B, VOCAB], f32r, tag="ex")
                sums = sp.tile([B, 1], f32, tag="sums")
                nc.scalar.activation(ex[:], lg[:], AFT.Exp, accum_out=sums[:])
                rr = sp.tile([B, 1], f32, tag="rr")
                nc.vector.reciprocal(rr[:], sums[:])
                dg = sp.tile([B, B], f32r, tag="dg")
                nc.vector.tensor_scalar_mul(dg[:], eye[:], rr[:])
                zxT = ps.tile([128, 4, B], f32, tag="zxT")
                for j in range(4):
                    nc.tensor.matmul(zxT[:, j, :], ex[:, 128 * j:128 * (j + 1)],
                                     dg[:], start=True, stop=True)
                xT = sp.tile([128, 4, B], f32r, tag="xT")
                nc.vector.tensor_copy(xT[:], zxT[:])

                # ---- G: finish g0(k+1) with wih0 @ xT -------------------
                for j in range(4):
                    nc.tensor.matmul(g0[:], xT[:, j, :], wih0[:, j, :],
                                     start=False, stop=(j == 3))

                # ---- H1: ew for layer 0 (step k+1) ----------------------
                h0 = ew(0, g0, c0)
                # ---- A1(k+1): start g1 partial (overlaps ew0) -----------
                g1 = ps.tile([B, GW], f32, tag="g1")
                nc.tensor.matmul(g1[:], ones[:], b1[:], start=True, stop=False)
                for j in range(4):
                    nc.tensor.matmul(g1[:], hg1[:, j, :], whh1[:, j, :],
                                     start=False, stop=False)
                # ---- H2: send h0, kick AG0(k+1) -------------------------
                ag0 = send(0, h0)
                # ---- A2(k+1): rest of whh1 (overlaps AG0) ---------------
                for j in range(4, N):
                    nc.tensor.matmul(g1[:], hg1[:, j, :], whh1[:, j, :],
                                     start=False, stop=False)

    nc.compile()
    return nc


T_STEPS = 512


def kernel(**inputs):
    import numpy as np
    from concourse import bass_utils
    in_maps = host_prep(**{k: np.asarray(v) for k, v in inputs.items()})
    nc = build_kernel(T_STEPS)
    res = bass_utils.run_bass_kernel_spmd(
        nc, in_maps, core_ids=list(range(N)), trace=False)
    logits = res.results[0]["out"]  # [T, B, V] fp32
    return np.ascontiguousarray(
        np.transpose(logits, (1, 0, 2))[:, :, None, :].astype(np.float32))


# revision 6
# speedup vs baseline: 1.0950x; 1.0375x over previous
"""CategoricalLstmDecoder Trainium kernel (self-contained).

8-way gate-split tensor parallel across 8 NeuronCores; bf16 matmuls
with f32 PSUM accumulation; h exchanged via AllGather in bf16.

Software-pipelined step: the whh matmuls of the next gate group are
issued while the AllGather of the other layer's h is in flight, so the
PE never sits idle across a collective. AllGather output lives in the
Shared scratchpad (fast HBM-HBM path).

Layout notes (per core c of 8):
- Gate chunk order per core: [i(128), f(128), o(128), g(128)] rows.
- h exchanged as h.T chunks [128 hdim, 64 batch] bf16 via ncfw AllGather.
"""
import sys
sys.path.insert(0, "/opt/trn_rl_repo")
import numpy as np
import concourse.bass as bass
import concourse.tile as tile
from concourse import bacc, mybir

N = 8
B = 64
HID = 1024
VOCAB = 512
CH = HID // N  # 128 h-dims per core
GW = 4 * CH    # 512 gate rows per core
f32 = mybir.dt.float32
bf16 = mybir.dt.bfloat16
AFT = mybir.ActivationFunctionType


def host_prep(z, fc_w, fc_b, w_ih0, w_hh0, b_ih0, b_hh0,
              w_ih1, w_hh1, b_ih1, b_hh1, out_w, out_b):
    """Build per-core input dicts. All fp32 numpy."""
    z = np.asarray(z, np.float32)
    h_init = np.tanh(z @ np.asarray(fc_w, np.float32).T + np.asarray(fc_b, np.float32))  # [B, HID]
    # hT layout [128, 8, 64]: [p, j, b] = h_init[b, 128*j + p]
    hT_init = np.transpose(h_init.reshape(B, N, CH), (2, 1, 0)).copy()  # [CH, N, B]

    b0 = np.asarray(b_ih0, np.float32) + np.asarray(b_hh0, np.float32)
    b1 = np.asarray(b_ih1, np.float32) + np.asarray(b_hh1, np.float32)

    def chunk_rows(c):
        # rows of the 4H gate matrix owned by core c, in [i, f, o, g] order
        i0 = np.arange(c * CH, (c + 1) * CH)
        return np.concatenate([i0, HID + i0, 3 * HID + i0, 2 * HID + i0])

    def prep_w(w, c):
        # returns W.T chunk [K, GW] as [128, K//128, GW]
        w = np.asarray(w, np.float32)
        rows = w[chunk_rows(c)]  # [GW, K]
        K = rows.shape[1]
        return np.ascontiguousarray(
            rows.T.reshape(K // 128, 128, GW).transpose(1, 0, 2))  # [128, K/128, GW]

    def prep_b(b, c):
        return b[chunk_rows(c)].astype(np.float32).reshape(1, GW).copy()

    out_wT = np.ascontiguousarray(
        np.asarray(out_w, np.float32).T.reshape(N, CH, VOCAB).transpose(1, 0, 2))  # [128, 8, 512]

    in_maps = []
    for c in range(N):
        in_maps.append({
            "hT_init": hT_init,
            "wih0": prep_w(w_ih0, c),
            "whh0": prep_w(w_hh0, c),
            "wih1": prep_w(w_ih1, c),
            "whh1": prep_w(w_hh1, c),
            "outw": out_wT,
            "b0": prep_b(b0, c),
            "b1": prep_b(b1, c),
            "outb": np.asarray(out_b, np.float32).reshape(1, VOCAB).copy(),
            "ones": np.ones((1, B), np.float32),
            "eye": np.eye(B, dtype=np.float32),
        })
    return in_maps


def build_kernel(T):
    nc = bacc.Bacc("TRN2", target_bir_lowering=False, debug=False, num_devices=N)
    dp = nc.declare_dram_parameter
    hT_init_d = dp("hT_init", [CH, N, B], f32, isOutput=False)
    wih0_d = dp("wih0", [128, 4, GW], f32, isOutput=False)
    whh0_d = dp("whh0", [128, 8, GW], f32, isOutput=False)
    wih1_d = dp("wih1", [128, 8, GW], f32, isOutput=False)
    whh1_d = dp("whh1", [128, 8, GW], f32, isOutput=False)
    outw_d = dp("outw", [128, 8, VOCAB], f32, isOutput=False)
    b0_d = dp("b0", [1, GW], f32, isOutput=False)
    b1_d = dp("b1", [1, GW], f32, isOutput=False)
    outb_d = dp("outb", [1, VOCAB], f32, isOutput=False)
    ones_d = dp("ones", [1, B], f32, isOutput=False)
    eye_d = dp("eye", [B, B], f32, isOutput=False)
    out_d = dp("out", [T, B, VOCAB], f32, isOutput=True)

    with tile.TileContext(nc) as tc:
        with (
            tc.tile_pool(name="wpool", bufs=1) as wpool,
            tc.tile_pool(name="state", bufs=1) as state,
            tc.tile_pool(name="sp", bufs=2) as sp,
            tc.tile_pool(name="ps", bufs=1, space=bass.MemorySpace.PSUM) as ps,
            tc.tile_pool(name="dram", bufs=2, space="DRAM") as dram,
        ):
            # ---- load + narrow weights to bf16 (one-time) ----
            def load_bf(dram_t, shape, name):
                stage = sp.tile(shape, f32, tag="wstage")
                nc.sync.dma_start(stage[:], dram_t[:])
                wr = wpool.tile(shape, bf16, tag=f"w_{name}")
                nc.vector.tensor_copy(wr[:], stage[:])
                return wr

            wih0 = load_bf(wih0_d, [128, 4, GW], "wih0")
            whh0 = load_bf(whh0_d, [128, 8, GW], "whh0")
            wih1 = load_bf(wih1_d, [128, 8, GW], "wih1")
            whh1 = load_bf(whh1_d, [128, 8, GW], "whh1")
            outw = load_bf(outw_d, [128, 8, VOCAB], "outw")
            b0 = load_bf(b0_d, [1, GW], "b0")
            b1 = load_bf(b1_d, [1, GW], "b1")
            outb = load_bf(outb_d, [1, VOCAB], "outb")
            ones = load_bf(ones_d, [1, B], "ones")
            eye = wpool.tile([B, B], f32, tag="eye")
            nc.sync.dma_start(eye[:], eye_d[:])
            eye_b = wpool.tile([B, B], bf16, tag="eye_b")
            nc.vector.tensor_copy(eye_b[:], eye[:])
            hTi = load_bf(hT_init_d, [CH, N, B], "hTi")  # full h_init.T (bf16)

            c0 = state.tile([B, CH], f32)
            c1 = state.tile([B, CH], f32)
            nc.vector.memset(c0[:], 0.0)
            nc.vector.memset(c1[:], 0.0)

            # ---- helpers -------------------------------------------------
            def ew(li, g, cstate):
                """gates psum g [B, GW] + cell state -> h sbuf tile [B, CH] bf16."""
                sg = sp.tile([B, 3 * CH], f32, tag=f"sg{li}")
                nc.scalar.activation(sg[:], g[:, 0:3 * CH], AFT.Sigmoid)
                tg = sp.tile([B, CH], f32, tag=f"tg{li}")
                nc.scalar.activation(tg[:], g[:, 3 * CH:GW], AFT.Tanh)
                t1 = sp.tile([B, CH], f32, tag=f"t1{li}")
                nc.vector.tensor_mul(t1[:], sg[:, 0:CH], tg[:])      # sig(i)*tanh(g)
                nc.vector.tensor_mul(cstate[:], cstate[:], sg[:, CH:2 * CH])  # c *= sig(f)
                nc.vector.tensor_add(cstate[:], cstate[:], t1[:])
                tc_ = sp.tile([B, CH], f32, tag=f"tc{li}")
                nc.scalar.activation(tc_[:], cstate[:], AFT.Tanh)
                h = sp.tile([B, CH], bf16, tag=f"h{li}")
                nc.vector.tensor_mul(h[:], sg[:, 2 * CH:3 * CH], tc_[:])  # sig(o)*tanh(c)
                return h

            def send(li, h):
                """transpose h -> bf16 chunk, DMA to DRAM, kick AllGather."""
                zhT = ps.tile([CH, B], bf16, tag=f"zhT{li}")
                nc.tensor.transpose(zhT[:], h[:], eye_b[:])
                hTs = sp.tile([CH, B], bf16, tag=f"hTs{li}")
                nc.vector.tensor_copy(hTs[:], zhT[:])
                cin = dram.tile([CH, B], bf16, tag=f"cin{li}")
                gout = dram.tile([N * CH, B], bf16, tag=f"gout{li}",
                                 addr_space="Shared")
                nc.sync.dma_start(cin[:], hTs[:])
                nc.gpsimd.collective_compute(
                    "AllGather", mybir.AluOpType.bypass,
                    replica_groups=[list(range(N))],
                    ins=[cin.opt()], outs=[gout.opt()],
                )
                return gout

            def recv(li, gout):
                """land the gathered h.T as a bf16 [CH, N, B] sbuf tile."""
                hg = sp.tile([CH, N, B], bf16, tag=f"hg{li}")
                src = gout.opt().rearrange("(c p) b -> p c b", p=CH)
                nc.sync.dma_start(hg[:], src[:])
                return hg

            # ---- prologue: step 0 layer 0 (x(0)=0, h0(-1)=h_init) -------
            g0 = ps.tile([B, GW], f32, tag="g0")
            nc.tensor.matmul(g0[:], ones[:], b0[:], start=True, stop=False)
            for j in range(N):
                nc.tensor.matmul(g0[:], hTi[:, j, :], whh0[:, j, :],
                                 start=False, stop=(j == N - 1))
            h0 = ew(0, g0, c0)
            ag0 = send(0, h0)
            # A(0): layer-1 recurrent part vs h1(-1)=h_init, overlaps AG0
            g1 = ps.tile([B, GW], f32, tag="g1")
            nc.tensor.matmul(g1[:], ones[:], b1[:], start=True, stop=False)
            for j in range(N):
                nc.tensor.matmul(g1[:], hTi[:, j, :], whh1[:, j, :],
                                 start=False, stop=False)

            for k in range(T):
                last = (k + 1 == T)
                # ---- B: land AG0(k); finish g1 with wih1 @ h0T(k) -------
                hg0 = recv(0, ag0)
                for j in range(N):
                    nc.tensor.matmul(g1[:], hg0[:, j, :], wih1[:, j, :],
                                     start=False, stop=(j == N - 1))
                # ---- C1: ew for layer 1 ---------------------------------
                h1 = ew(1, g1, c1)
                # ---- D1: start g0(k+1) partial (overlaps ew1) -----------
                if not last:
                    g0 = ps.tile([B, GW], f32, tag="g0")
                    nc.tensor.matmul(g0[:], ones[:], b0[:], start=True, stop=False)
                    for j in range(4):
                        nc.tensor.matmul(g0[:], hg0[:, j, :], whh0[:, j, :],
                                         start=False, stop=False)
                # ---- C2: send h1, kick AG1(k) ---------------------------
                ag1 = send(1, h1)
                # ---- D2: rest of whh0 (overlaps AG1) --------------------
                if not last:
                    for j in range(4, N):
                        nc.tensor.matmul(g0[:], hg0[:, j, :], whh0[:, j, :],
                                         start=False, stop=False)
                # ---- E: land AG1(k); logits -----------------------------
                hg1 = recv(1, ag1)
                lg = ps.tile([B, VOCAB], f32, tag="lg")
                nc.tensor.matmul(lg[:], ones[:], outb[:], start=True, stop=False)
                for j in range(N):
                    nc.tensor.matmul(lg[:], hg1[:, j, :], outw[:, j, :],
                                     start=False, stop=(j == N - 1))
                lgs = sp.tile([B, VOCAB], f32, tag="lgs")
                nc.vector.tensor_copy(lgs[:], lg[:])
                nc.gpsimd.dma_start(out_d[k], lgs[:])

                if last:
                    break

                # ---- F: softmax -> xT (bf16 [128, 4, B]) ----------------
                ex = sp.tile([B, VOCAB], bf16, tag="ex")
                sums = sp.tile([B, 1], f32, tag="sums")
                nc.scalar.activation(ex[:], lg[:], AFT.Exp, accum_out=sums[:])
                rr = sp.tile([B, 1], f32, tag="rr")
                nc.vector.reciprocal(rr[:], sums[:])
                dg = sp.tile([B, B], bf16, tag="dg")
                nc.vector.tensor_scalar_mul(dg[:], eye[:], rr[:])
                zxT = ps.tile([128, 4, B], f32, tag="zxT")
                for j in range(4):
                    nc.tensor.matmul(zxT[:, j, :], ex[:, 128 * j:128 * (j + 1)],
                                     dg[:], start=True, stop=True)
                xT = sp.tile([128, 4, B], bf16, tag="xT")
                nc.vector.tensor_copy(xT[:], zxT[:])

                # ---- G: finish g0(k+1) with wih0 @ xT -------------------
                for j in range(4):
                    nc.tensor.matmul(g0[:], xT[:, j, :], wih0[:, j, :],
                                     start=False, stop=(j == 3))

                # ---- H1: ew for layer 0 (step k+1) ----------------------
                h0 = ew(0, g0, c0)
                # ---- A1(k+1): start g1 partial (overlaps ew0) -----------
                g1 = ps.tile([B, GW], f32, tag="g1")
                nc.tensor.matmul(g1[:], ones[:], b1[:], start=True, stop=False)
                for j in range(4):
                    nc.tensor.matmul(g1[:], hg1[:, j, :], whh1[:, j, :],
                                     start=False, stop=False)
                # ---- H2: send h0, kick AG0(k+1) -------------------------
                ag0 = send(0, h0)
                # ---- A2(k+1): rest of whh1 (overlaps AG0) ---------------
                for j in range(4, N):
                    nc.tensor.matmul(g1[:], hg1[:, j, :], whh1[:, j, :],
                                     start=False, stop=False)

    nc.compile()
    return nc


T_STEPS = 512


def kernel(**inputs):
    import numpy as np
    from concourse import bass_utils
    in_maps = host_prep(**{k: np.asarray(v) for k, v in inputs.items()})
    nc = build_kernel(T_STEPS)
    res = bass_utils.run_bass_kernel_spmd(
        nc, in_maps, core_ids=list(range(N)), trace=False)
    logits = res.results[0]["out"]  # [T, B, V] fp32
    return np.ascontiguousarray(
        np.transpose(logits, (1, 0, 2))[:, :, None, :].astype(np.float32))
